# revision 23
# baseline (speedup 1.0000x reference)
"""Trainium2 Bass kernel for nn_ChiralEmbeddingModel (chiral tensor-product embedding).

Math (per atom n, with x = atomic_embeddings[n, 256:].reshape(128, 3)):
    ms   = mean(x^2)                          (over all 384 components)
    s    = 1/sqrt(ms + eps)
    chi  = C1*C2 * s^3 * [x . (g w2 (x_g x  cross  g w1 x))]   (g folded into weights)
    out  = s^3 * (chi_raw @ Wo') + b

Key restructure vs v1:
  * x is pre-transposed to mul-major [3, 128(mul), atoms] fp16 tiles on the
    HOST, so the device needs no PE transposes, no diag build, and no xt
    eviction.
  * The RMS normalization is algebraically s^3 * chi_raw; s^3 and the bias
    are applied on the HOST to the fp16 output, so the device computes the
    raw chain only (no stats pass on device).
  * Everything on-device is fp16 (PE 1 cyc/row, DVE 2x mode) with fp32 PSUM.

Per 512-atom tile:
    y   = w1' @ x        (3 matmuls, PSUM f32)  -> ACT evicts to fp16 SBUF
    bp  = x_a * y_b      (2 batched DVE products at 2x via the 5-slot x
                          layout [x0,x1,x2,x0,x1] prepared on host)
    z   = +-w2' @ bp     (6 matmuls, accumulated pairs in PSUM)
    call= x * z          (1 batched DVE product, PSUM-read 1x)
    chi = call0+call1+call2 (2 DVE adds)
    out = chi.T @ Wo'    (4 matmuls) -> ACT evicts fp16 -> DMA

Sharding: pure data-parallel over the atom axis across 8 NeuronCores; small
weights replicated.
"""

import numpy as np

N_TOTAL = 131072
N_CORES = 8
N_SHARD = N_TOTAL // N_CORES  # 16384
INV = 256
MUL = 128
EDIM = 3
OUT = 512
EPS = 1e-6
C1 = (3.0 / 256.0) ** 0.5
C2 = (1.0 / 384.0) ** 0.5
P = 128
TILE = 512
NCHUNK = TILE // P  # 4

# cross product index pairs: cross_0 = x1*y2 - x2*y1, etc.
PLUS = [(1, 2), (2, 0), (0, 1)]
MINUS = [(2, 1), (0, 2), (1, 0)]


def _build_nc(n_shard: int, loop_repeat: int = 1, fold_chi: int = 0,
              y_evict: str = "scalar", out_evict: str = "scalar",
              cp_evict: int = 0, dup: int = 1, ablate: str = "",
              s1: int = 0, s2: int = 0, s3: int = 0, bufs: int = 4,
              xt_bufs: int = 5, chi_engine: str = "vector"):
    import contextlib

    import concourse.bacc as bacc
    import concourse.tile as tile
    from concourse import mybir

    f32 = mybir.dt.float32
    f16 = mybir.dt.float16

    assert n_shard % TILE == 0
    n_tiles = n_shard // TILE

    nc = bacc.Bacc("TRN2", target_bir_lowering=False, debug=False)

    xt_d = nc.dram_tensor("xt", [n_tiles, 5, P, TILE], f16,
                          kind="ExternalInput").ap()
    w1t = nc.dram_tensor("w1t", [MUL, MUL], f16, kind="ExternalInput").ap()
    w2pt = nc.dram_tensor("w2pt", [MUL, MUL], f16, kind="ExternalInput").ap()
    w2mt = nc.dram_tensor("w2mt", [MUL, MUL], f16, kind="ExternalInput").ap()
    wot = nc.dram_tensor("wot", [MUL, OUT], f16, kind="ExternalInput").ap()
    out_d = nc.dram_tensor("out", [n_shard, OUT], f16, kind="ExternalOutput").ap()
    out_t = out_d.rearrange("(t c p) o -> t c p o", c=NCHUNK, p=P)

    with tile.TileContext(nc) as tc:
        with (
            tc.tile_pool(name="singles", bufs=1) as singles,
            tc.tile_pool(name="xt", bufs=xt_bufs) as xt_pool,
            tc.tile_pool(name="y16", bufs=bufs) as y16_pool,
            tc.tile_pool(name="bp", bufs=bufs) as bp_pool,
            tc.tile_pool(name="call", bufs=bufs) as call_pool,
            tc.tile_pool(name="chi", bufs=bufs) as chi_pool,
            tc.tile_pool(name="o16", bufs=bufs) as o16_pool,
            tc.tile_pool(name="psY", bufs=1, space="PSUM") as psY,
            tc.tile_pool(name="psZ", bufs=1, space="PSUM") as psZ,
            tc.tile_pool(name="psO", bufs=1, space="PSUM") as psO,
        ):
            w1t_sb = singles.tile([MUL, MUL], f16)
            w2pt_sb = singles.tile([MUL, MUL], f16)
            w2mt_sb = singles.tile([MUL, MUL], f16)
            wot_sb = singles.tile([MUL, OUT], f16)
            nc.sync.dma_start(out=w1t_sb, in_=w1t)
            nc.sync.dma_start(out=w2pt_sb, in_=w2pt)
            nc.sync.dma_start(out=w2mt_sb, in_=w2mt)
            nc.sync.dma_start(out=wot_sb, in_=wot)

            # ---- software-pipelined stages; state[j] holds tile j's tiles
            state = {}

            def st_load(j):
                # 5-slot layout [x0,x1,x2,x0,x1] prepared on host: both
                # product groups below are contiguous 3-slot views
                xt_all = xt_pool.tile([P, 5, TILE], f16, tag="xt")
                nc.sync.dma_start(
                    out=xt_all, in_=xt_d[j].rearrange("j u a -> u j a")
                )
                state[j] = {"xt": xt_all}

            def st_y(j):
                xt_all = state[j]["xt"]
                y_ps = psY.tile([P, EDIM, TILE], f32, tag="y")
                for k in range(EDIM):
                    nc.tensor.matmul(
                        y_ps[:, k, :], w1t_sb, xt_all[:, k, :],
                        start=True, stop=True,
                    )
                state[j]["y_ps"] = y_ps

            def st_yev(j):
                y16 = y16_pool.tile([P, EDIM, TILE], f16, tag="y16")
                if y_evict == "scalar":
                    nc.scalar.copy(y16, state[j]["y_ps"])
                else:
                    nc.vector.tensor_copy(y16, state[j]["y_ps"])
                state[j]["y16"] = y16

            def st_bp(j):
                xt_all, y16 = state[j]["xt"], state[j]["y16"]
                bp = bp_pool.tile([P, 2 * EDIM, TILE], f16, tag="bp")
                if dup:
                    # bp_plus[i] = x_{(i+2)%3} * y_i = xt[2:5] * y16
                    # bp_minus[i] = x_{(i+1)%3} * y_i = xt[1:4] * y16
                    nc.vector.tensor_mul(
                        bp[:, 0:EDIM, :], xt_all[:, 2:5, :], y16
                    )
                    nc.vector.tensor_mul(
                        bp[:, EDIM:2 * EDIM, :], xt_all[:, 1:4, :], y16
                    )
                else:
                    for k, (a, b) in enumerate(PLUS + MINUS):
                        nc.vector.tensor_mul(
                            bp[:, k, :], xt_all[:, a, :], y16[:, b, :]
                        )
                state[j]["bp"] = bp

            def st_z(j):
                bp = state[j]["bp"]
                z_ps = psZ.tile([P, EDIM, TILE], f32, tag="z")
                # slot holding bp_plus[i] / bp_minus[i] (dup layout rotates)
                pslot = (lambda i: (i + 2) % 3) if dup else (lambda i: i)
                mslot = (lambda i: EDIM + (i + 1) % 3) if dup else (
                    lambda i: EDIM + i)
                for i in range(EDIM):
                    nc.tensor.matmul(
                        z_ps[:, i, :], w2pt_sb, bp[:, pslot(i), :],
                        start=True, stop=False,
                    )
                for i in range(EDIM):
                    nc.tensor.matmul(
                        z_ps[:, i, :], w2mt_sb, bp[:, mslot(i), :],
                        start=False, stop=True,
                    )
                state[j]["z_ps"] = z_ps

            def st_cp(j):
                xt_all, z_ps = state[j]["xt"], state[j]["z_ps"]
                call = call_pool.tile([P, EDIM, TILE], f16, tag="call")
                nc.vector.tensor_mul(call, xt_all[:, 0:EDIM, :], z_ps)
                if fold_chi:
                    chi2 = chi_pool.tile([P, TILE], f16, tag="chi")
                    nc.vector.tensor_add(chi2, call[:, 0, :], call[:, 1, :])
                    state[j]["chi_parts"] = [chi2, call[:, 2, :]]
                else:
                    eng = nc.gpsimd if chi_engine == "gpsimd" else nc.vector
                    chi01 = chi_pool.tile([P, TILE], f16, tag="chi")
                    eng.tensor_add(chi01, call[:, 0, :], call[:, 1, :])
                    chi = chi_pool.tile([P, TILE], f16, tag="chi")
                    eng.tensor_add(chi, chi01, call[:, 2, :])
                    state[j]["chi_parts"] = [chi]
                state[j]["call"] = call

            def st_o(j):
                chi_parts = state[j]["chi_parts"]
                o16 = o16_pool.tile([P, NCHUNK, OUT], f16, tag="o16")
                for q in range(NCHUNK // 2):
                    o_ps = psO.tile([P, 2, OUT], f32, tag="o")
                    for h in range(2):
                        c = 2 * q + h
                        for pi, part in enumerate(chi_parts):
                            nc.tensor.matmul(
                                o_ps[:, h, :],
                                part[:, c * P:(c + 1) * P],
                                wot_sb,
                                start=(pi == 0),
                                stop=(pi == len(chi_parts) - 1),
                            )
                    if out_evict == "scalar":
                        nc.scalar.copy(o16[:, 2 * q:2 * q + 2, :], o_ps)
                    else:
                        nc.vector.tensor_copy(o16[:, 2 * q:2 * q + 2, :], o_ps)
                nc.sync.dma_start(
                    out=out_t[j].rearrange("c p o -> p c o"), in_=o16
                )
                # tile j fully consumed; allow python to drop references
                del state[j]

            loop_cm = (
                tc.For_i(0, loop_repeat, 1)
                if loop_repeat > 1
                else contextlib.nullcontext()
            )
            with loop_cm:
             for i in range(n_tiles + max(s1, s2, s3)):
                if i < n_tiles:
                    st_load(i)
                    st_y(i)
                    st_yev(i)
                if 0 <= i - s1 < n_tiles:
                    st_bp(i - s1)
                if 0 <= i - s2 < n_tiles:
                    st_z(i - s2)
                    st_cp(i - s2)
                if 0 <= i - s3 < n_tiles:
                    st_o(i - s3)

    nc.finalize()
    return nc


def _host_prep(inputs):
    emb = np.asarray(inputs["atomic_embeddings"], dtype=np.float32)
    g = np.asarray(inputs["rms_g"], dtype=np.float32)
    w1 = np.asarray(inputs["w1"], dtype=np.float32)
    w2 = np.asarray(inputs["w2"], dtype=np.float32)
    W_out = np.asarray(inputs["W_out"], dtype=np.float32)
    b_out = np.asarray(inputs["b_out"], dtype=np.float32)

    x = np.ascontiguousarray(emb[:, INV:])            # [N, 384]
    n = x.shape[0]
    ms = np.einsum("nf,nf->n", x, x) / np.float32(MUL * EDIM)
    s3 = (ms + np.float32(EPS)) ** np.float32(-1.5)   # [N]

    # [N, 384] -> [T, 512, 128, 3] -> [T, 3, 128, 512] fp16, then pad to the
    # 5-slot layout [x0, x1, x2, x0, x1] (makes both device-side product
    # groups contiguous 3-slot views)
    xt3 = x.reshape(n // TILE, TILE, MUL, EDIM).transpose(0, 3, 2, 1)
    xt = np.ascontiguousarray(
        np.concatenate([xt3, xt3[:, 0:2]], axis=1)
    ).astype(np.float16)

    consts = {
        "w1t": np.ascontiguousarray(C1 * (w1.T * g[:, None])).astype(np.float16),
        "w2pt": np.ascontiguousarray(C2 * (w2.T * g[:, None])).astype(np.float16),
        "w2mt": np.ascontiguousarray(-C2 * (w2.T * g[:, None])).astype(np.float16),
        "wot": np.ascontiguousarray(W_out.T * g[:, None]).astype(np.float16),
    }
    return xt, consts, s3, b_out


_NC_CACHE = {}


def _get_nc(n_shard):
    if n_shard not in _NC_CACHE:
        _NC_CACHE[n_shard] = _build_nc(n_shard)
    return _NC_CACHE[n_shard]


def _postproc(res, s3, b_out):
    out16 = np.concatenate(
        [np.asarray(res.results[i]["out"]) for i in range(N_CORES)], axis=0
    )
    return (out16.astype(np.float32) * s3[:, None] + b_out[None, :]).astype(
        np.float32
    )


def _in_maps(xt, consts):
    t_shard = N_SHARD // TILE
    in_maps = []
    for i in range(N_CORES):
        m = {"xt": xt[i * t_shard:(i + 1) * t_shard]}
        m.update(consts)
        in_maps.append(m)
    return in_maps


def kernel(**inputs) -> np.ndarray:
    from concourse.bass_utils import run_bass_kernel_spmd

    xt, consts, s3, b_out = _host_prep(inputs)
    assert xt.shape[0] * TILE == N_TOTAL, f"expected {N_TOTAL} atoms"

    nc = _get_nc(N_SHARD)
    res = run_bass_kernel_spmd(nc, _in_maps(xt, consts), list(range(N_CORES)))
    return _postproc(res, s3, b_out)
